# revision 23
# baseline (speedup 1.0000x reference)
"""Trainium2 Bass kernel for nn_BlockCorrelation (sparse/block attention).

Self-contained: accepts FULL inputs, shards across 8 NeuronCores internally,
returns the FULL output.

Math (see reference):
    feat = x.mean((2,3)); feat_n = LN(feat)*ln_w + ln_b
    qkv  = feat_n @ in_proj_w.T + in_proj_b  -> 8-head attention with a
    block-diagonal mask (attend only within equal batch_indices groups)
    out  = attn_out @ out_proj_w.T + out_proj_b
    y    = x + where(group_count>1, gamma*out, 0)[..., None, None]

Distribution / pipelining:
  - rows are sorted by group on the host; core k owns sorted row tiles k and
    k+8 (interleaved). Pooling/LN and the elementwise passes are data-parallel
    over rows; attention is tensor-parallel over heads (core k = head k).
  - per-half pipeline: pool half A -> AllGather featT(A) -> QKV(A) ->
    banded scores/av for query chunks 0-1 -> out_proj partial(A) ->
    ReduceScatter(A) -> the y = x + g*delta pass for half-A rows STREAMS
    while half B's attention and ReduceScatter are still in flight.
  - the sorted band limits each 512-query chunk to key tiles [4j-1, 4j+4]
    (valid while every group fits in one 128 tile; dense fallback otherwise).

Layout: attention tensors are transposed (contraction on partitions); the
softmax denominator is a ones-matmul (no partition reduction); the group mask
is added inside the scores PSUM via a rank-32 one-hot matmul
(-50*(1-same)), so exp() zeroes out-of-group entries.  Scores are O(1)
(LN'd features, 0.02-scale weights), so no max-subtraction is needed.
"""

import json
import sys

if "/opt/trn_rl_repo" not in sys.path:
    sys.path.insert(0, "/opt/trn_rl_repo")

import ml_dtypes
import numpy as np

import concourse.bass as bass
import concourse.mybir as mybir
import concourse.tile as tile
from concourse.bass_utils import run_bass_kernel_spmd
from concourse.masks import make_identity

F32 = mybir.dt.float32
BF16 = mybir.dt.bfloat16

# Problem shape (hardcoded per contract)
N, C, HW = 2048, 1024, 64
NH, HD = 8, 128
NG = 32
EPS = 1e-5
NCORES = 8
NS = N // NCORES          # 256 rows per core
NT = 2                    # halves (core row-tiles per phase)
CB = C // 128             # 8 channel blocks
MT = N // 128             # 16 global key tiles
NQC = N // 512            # 4 query chunks of 512
HMT = MT // 2             # 8 key tiles per half
MASK_NEG = -50.0          # additive mask magnitude (e^-50 ~ 2e-22)
GCAP = 128                # band attention assumes group fits in one m-tile


def _band(j, dense):
    if dense:
        return range(MT)
    return range(max(0, 4 * j - 1), min(MT, 4 * j + 5))


# ---------------------------------------------------------------------------
# walrus workaround: this build rejects >1 sem wait per instruction in some
# CTRL lowerings; split excess on_wait entries onto preceding same-engine
# EventSemaphore instructions (the exact shape wait_ge() lowers to).
def _split_waits_json(j, max_waits=1):
    for f in j.get("functions", []):
        for bb in f.get("blocks", []):
            out = []
            for ins in bb.get("instructions", []):
                si = ins.get("sync_info")
                waits = (si or {}).get("on_wait") or []
                if len(waits) > max_waits:
                    head, tail = waits[:-max_waits], waits[-max_waits:]
                    for k, w in enumerate(head):
                        out.append({
                            "name": f"{ins['name']}-wsplit{k}",
                            "opcode": "EventSemaphore",
                            "engine": ins["engine"],
                            "ins": [],
                            "outs": [],
                            "debug": ins.get("debug", 0),
                            "sync_info": {"on_update": [], "on_wait": [w]},
                        })
                    si["on_wait"] = tail
                out.append(ins)
            bb["instructions"] = out
    return j


def _install_wait_split(nc, max_waits=1):
    def to_json_bytes_fixed():
        j = json.loads(mybir.module_to_json_bytes(nc.m))
        return json.dumps(_split_waits_json(j, max_waits)).encode()

    nc.to_json_bytes = to_json_bytes_fixed


def _bcast_ap(ap, parts=128):
    """DRAM AP broadcast across partitions (stride-0 partition dim)."""
    return bass.AP(tensor=ap.tensor, offset=ap.offset, ap=[[0, parts]] + ap.ap)


# ---------------------------------------------------------------------------
def build_program(dense=False):
    nc = bass.Bass(num_devices=NCORES)

    # --- per-core parameters (SPMD: same program, different data) ---
    # xs rows: [sorted tile k (128 rows), sorted tile k+8 (128 rows)]
    xs = nc.declare_dram_parameter("xs", [NS, C, HW], F32, isOutput=False)
    wq = nc.declare_dram_parameter("wq", [C, HD], BF16, isOutput=False)
    wk = nc.declare_dram_parameter("wk", [C, HD], BF16, isOutput=False)
    wv = nc.declare_dram_parameter("wv", [C, HD], BF16, isOutput=False)
    qb = nc.declare_dram_parameter("qb", [HD, 1], F32, isOutput=False)
    kb = nc.declare_dram_parameter("kb", [HD, 1], F32, isOutput=False)
    vb = nc.declare_dram_parameter("vb", [1, HD], BF16, isOutput=False)
    wo = nc.declare_dram_parameter("wo", [HD, C], BF16, isOutput=False)
    oha = nc.declare_dram_parameter("oha", [NG, N], BF16, isOutput=False)
    ohb = nc.declare_dram_parameter("ohb", [NG, N], BF16, isOutput=False)
    lnw = nc.declare_dram_parameter("lnw", [C], F32, isOutput=False)
    lnb = nc.declare_dram_parameter("lnb", [C], F32, isOutput=False)
    bo = nc.declare_dram_parameter("bo", [C], F32, isOutput=False)
    gsh = nc.declare_dram_parameter("gsh", [NS], F32, isOutput=False)
    out = nc.declare_dram_parameter("out", [NS, C, HW], F32, isOutput=True)

    # --- internal DRAM for collectives (per half) ---
    featT_sh = [nc.dram_tensor(f"featT_sh{h}", [C, 128], BF16) for h in range(NT)]
    featT_full = [
        nc.dram_tensor(f"featT_full{h}", [NCORES * C, 128], BF16,
                       addr_space="Shared")
        for h in range(NT)
    ]
    delta_part = [nc.dram_tensor(f"delta_part{h}", [N // 2, C], F32)
                  for h in range(NT)]
    delta_rs = [nc.dram_tensor(f"delta_rs{h}", [128, C], F32) for h in range(NT)]

    groups = [list(range(NCORES))]
    inv_sqrt_hd = 1.0 / float(np.sqrt(np.float32(HD)))

    with tile.TileContext(nc, num_cores=NCORES) as tc:
        with tc.tile_pool(name="singles", bufs=1) as singles:
            ident = singles.tile([128, 128], F32)
            make_identity(nc, ident)
            ones_col = singles.tile([128, 1], BF16)
            nc.vector.memset(ones_col, 1.0)
            one_1x1 = singles.tile([1, 1], F32)
            nc.vector.memset(one_1x1, 1.0)
            ones_row = singles.tile([1, 128], BF16)
            nc.vector.memset(ones_row, 1.0)

            # ------------- phase 1: pool + LN + transpose + AG per half -------------
            with (
                tc.tile_pool(name="xin", bufs=3) as xin,
                tc.tile_pool(name="p1", bufs=2) as p1,
                tc.tile_pool(name="p1one", bufs=1) as p1one,
                tc.tile_pool(name="p1ps", bufs=2, space="PSUM") as p1ps,
            ):
                eps_t = p1one.tile([128, 1], F32)
                nc.vector.memset(eps_t, EPS * HW * HW)  # LN on sums: eps * 64^2
                lnw_t = p1one.tile([128, C], F32)
                nc.gpsimd.dma_start(out=lnw_t, in_=_bcast_ap(lnw[:]))
                lnb_t = p1one.tile([128, C], F32)
                nc.gpsimd.dma_start(out=lnb_t, in_=_bcast_ap(lnb[:]))

                for h in range(NT):
                    fsum = p1.tile([128, C], F32, tag="fsum")
                    for cc in range(CB):
                        xt = xin.tile([128, 128, HW], F32, tag="xt")
                        eng = nc.sync if cc % 2 == 0 else nc.scalar
                        eng.dma_start(
                            out=xt,
                            in_=xs[h * 128:(h + 1) * 128,
                                   cc * 128:(cc + 1) * 128, :])
                        nc.vector.reduce_sum(
                            out=fsum[:, cc * 128:(cc + 1) * 128],
                            in_=xt, axis=mybir.AxisListType.X)
                    stats = p1.tile([128, 2, 6], F32, tag="stats")
                    for sg in range(2):
                        nc.vector.bn_stats(out=stats[:, sg, :],
                                           in_=fsum[:, sg * 512:(sg + 1) * 512])
                    mv = p1.tile([128, 2], F32, tag="mv")
                    nc.vector.bn_aggr(out=mv, in_=stats)
                    std = p1.tile([128, 1], F32, tag="std")
                    nc.scalar.activation(
                        out=std, in_=mv[:, 1:2],
                        func=mybir.ActivationFunctionType.Sqrt, bias=eps_t, scale=1.0)
                    rstd = p1.tile([128, 1], F32, tag="rstd")
                    nc.vector.reciprocal(out=rstd, in_=std)
                    featn = p1.tile([128, C], F32, tag="featn")
                    nc.vector.tensor_scalar(
                        out=featn, in0=fsum, scalar1=mv[:, 0:1], scalar2=rstd,
                        op0=mybir.AluOpType.subtract, op1=mybir.AluOpType.mult)
                    nc.vector.tensor_mul(out=featn, in0=featn, in1=lnw_t)
                    nc.vector.tensor_add(out=featn, in0=featn, in1=lnb_t)
                    fTb = p1.tile([128, CB, 128], BF16, tag="fTb")
                    for cbi in range(CB):
                        pt = p1ps.tile([128, 128], F32, tag="trps")
                        nc.tensor.transpose(pt, featn[:, cbi * 128:(cbi + 1) * 128],
                                            ident)
                        nc.vector.tensor_copy(out=fTb[:, cbi, :], in_=pt)
                    nc.sync.dma_start(
                        out=featT_sh[h].rearrange("(cb p) n -> p cb n", p=128),
                        in_=fTb)
                    nc.gpsimd.collective_compute(
                        "AllGather", mybir.AluOpType.bypass, replica_groups=groups,
                        ins=[featT_sh[h][:]], outs=[featT_full[h][:]])

            # ---------------- phase 2+3: per-half pipeline ----------------
            with (
                tc.tile_pool(name="attper", bufs=1) as attper,
                tc.tile_pool(name="qkvw", bufs=1) as qkvw,
                tc.tile_pool(name="maskp", bufs=1) as maskp,
                tc.tile_pool(name="expbuf", bufs=2) as expbuf,
                tc.tile_pool(name="dsbp", bufs=2) as dsbp,
                tc.tile_pool(name="p3", bufs=1) as p3,
                tc.tile_pool(name="ftp", bufs=1) as ftp,
                tc.tile_pool(name="xin3", bufs=6) as xin3,
                tc.tile_pool(name="qkvps", bufs=2, space="PSUM") as qkvps,
                tc.tile_pool(name="scoreps", bufs=3, space="PSUM") as scoreps,
                tc.tile_pool(name="avdps", bufs=2, space="PSUM") as avdps,
                tc.tile_pool(name="denps", bufs=1, space="PSUM") as denps,
            ):
                # head weights + onehots (tiny; load once)
                wq_t = qkvw.tile([128, CB, HD], BF16)
                nc.gpsimd.dma_start(out=wq_t,
                                  in_=wq.rearrange("(cb p) d -> p cb d", p=128))
                wk_t = qkvw.tile([128, CB, HD], BF16)
                nc.gpsimd.dma_start(out=wk_t,
                                  in_=wk.rearrange("(cb p) d -> p cb d", p=128))
                wv_t = qkvw.tile([128, CB, HD], BF16)
                nc.gpsimd.dma_start(out=wv_t,
                                  in_=wv.rearrange("(cb p) d -> p cb d", p=128))
                qb_t = qkvw.tile([128, 1], F32)
                nc.gpsimd.dma_start(out=qb_t, in_=qb[:])
                kb_t = qkvw.tile([128, 1], F32)
                nc.gpsimd.dma_start(out=kb_t, in_=kb[:])
                vb_t = qkvw.tile([1, 128], BF16)
                nc.gpsimd.dma_start(out=vb_t, in_=vb[:])
                wo_t = qkvw.tile([128, C], BF16)
                nc.gpsimd.dma_start(out=wo_t, in_=wo[:])
                oha_t = maskp.tile([128, N], BF16)
                nc.vector.memset(oha_t, 0.0)
                nc.gpsimd.dma_start(out=oha_t[:NG, :], in_=oha[:])
                ohb_t = maskp.tile([128, N], BF16)
                nc.vector.memset(ohb_t, 0.0)
                nc.gpsimd.dma_start(out=ohb_t[:NG, :], in_=ohb[:])
                bo_t = p3.tile([128, C], F32)
                nc.gpsimd.dma_start(out=bo_t, in_=_bcast_ap(bo[:]))
                g_t = p3.tile([128, NT], F32)
                nc.gpsimd.dma_start(out=g_t, in_=gsh.rearrange("(t p) -> p t", p=128))

                # per-half feature/QKV tensors (global n = 1024*h + 128*co + nl)
                ft = {}
                qT = [attper.tile([128, N // 2], BF16, tag=f"qT{h}", name=f"qT{h}")
                      for h in range(NT)]
                kT = [attper.tile([128, N // 2], BF16, tag=f"kT{h}", name=f"kT{h}")
                      for h in range(NT)]
                v_t = [attper.tile([128, HMT, HD], BF16, tag=f"v{h}", name=f"v{h}")
                       for h in range(NT)]
                avT = [attper.tile([128, N // 2], BF16, tag=f"avT{h}", name=f"avT{h}")
                       for h in range(NT)]
                den_row = [attper.tile([1, N // 2], F32, tag=f"den{h}", name=f"den{h}")
                           for h in range(NT)]
                recipT = [attper.tile([128, HMT], F32, tag=f"recipT{h}", name=f"recipT{h}")
                          for h in range(NT)]

                def load_ft(h):
                    ft[h] = ftp.tile([128, CB, N // 2], BF16, tag="ft", name="ft")
                    ftv = featT_full[h].rearrange(
                        "(co cb p) n -> cb p co n", co=NCORES, p=128)
                    for cbi in range(CB):
                        nc.gpsimd.dma_start(
                            out=ft[h][:, cbi, :].rearrange(
                                "p (co nl) -> p co nl", co=NCORES),
                            in_=ftv[cbi])

                def qkv_half(h):
                    for jj in range(2):
                        sl = slice(jj * 512, (jj + 1) * 512)
                        pq = qkvps.tile([128, 512], F32, tag="pqkv")
                        for cbi in range(CB):
                            nc.tensor.matmul(pq, wq_t[:, cbi, :], ft[h][:, cbi, sl],
                                             start=(cbi == 0), stop=(cbi == CB - 1))
                        nc.vector.tensor_scalar(
                            out=qT[h][:, sl], in0=pq, scalar1=qb_t,
                            scalar2=inv_sqrt_hd,
                            op0=mybir.AluOpType.add, op1=mybir.AluOpType.mult)
                        pk = qkvps.tile([128, 512], F32, tag="pqkv")
                        for cbi in range(CB):
                            nc.tensor.matmul(pk, wk_t[:, cbi, :], ft[h][:, cbi, sl],
                                             start=(cbi == 0), stop=(cbi == CB - 1))
                        nc.vector.tensor_scalar(
                            out=kT[h][:, sl], in0=pk, scalar1=kb_t, scalar2=None,
                            op0=mybir.AluOpType.add)
                    for lt in range(HMT):
                        msl = slice(lt * 128, (lt + 1) * 128)
                        pv = qkvps.tile([128, HD], F32, tag="pqkv")
                        for cbi in range(CB):
                            nc.tensor.matmul(pv, ft[h][:, cbi, msl], wv_t[:, cbi, :],
                                             start=(cbi == 0), stop=False)
                        nc.tensor.matmul(pv, ones_row, vb_t, start=False, stop=True)
                        nc.scalar.activation(out=v_t[h][:, lt, :], in_=pv,
                                             func=mybir.ActivationFunctionType.Copy)

                BW = MT if dense else 6

                def attn_chunk(j):
                    """scoresT band -> exp -> av + denominator for query chunk j."""
                    h = j // 2
                    sl = slice((j % 2) * 512, (j % 2 + 1) * 512)
                    gsl = slice(j * 512, (j + 1) * 512)  # global (onehot) cols
                    band = list(_band(j, dense))
                    expj = expbuf.tile([128, BW, 512], BF16, tag="expj")
                    pav = avdps.tile([128, 512], F32, tag="pav")
                    pden = denps.tile([1, 512], F32, tag="pden")
                    for bi_, mt in enumerate(band):
                        mh, lt = mt // HMT, mt % HMT
                        msl = slice(lt * 128, (lt + 1) * 128)
                        gmsl = slice(mt * 128, (mt + 1) * 128)
                        ps = scoreps.tile([128, 512], F32, tag="pscore")
                        nc.tensor.matmul(ps, kT[mh][:, msl], qT[h][:, sl],
                                         start=True, stop=False)
                        nc.tensor.matmul(ps, oha_t[:, gmsl], ohb_t[:, gsl],
                                         start=False, stop=True)
                        nc.scalar.activation(
                            out=expj[:, bi_, :], in_=ps,
                            func=mybir.ActivationFunctionType.Exp)
                    nb = len(band)
                    for bi_, mt in enumerate(band):
                        mh, lt = mt // HMT, mt % HMT
                        nc.tensor.matmul(pav, v_t[mh][:, lt, :], expj[:, bi_, :],
                                         start=(bi_ == 0), stop=(bi_ == nb - 1))
                    for bi_, mt in enumerate(band):
                        nc.tensor.matmul(pden, ones_col, expj[:, bi_, :],
                                         start=(bi_ == 0), stop=(bi_ == nb - 1))
                    nc.vector.tensor_copy(out=avT[h][:, sl], in_=pav)
                    nc.vector.tensor_copy(out=den_row[h][:, sl], in_=pden)

                def finish_half(h):
                    """denominators -> out_proj partial -> ReduceScatter."""
                    denT = attper.tile([128, HMT], F32, tag=f"denT{h}", name=f"denT{h}")
                    for lt in range(HMT):
                        pdt = qkvps.tile([128, 1], F32, tag="pqkv")
                        nc.tensor.matmul(pdt,
                                         den_row[h][0:1, lt * 128:(lt + 1) * 128],
                                         one_1x1, start=True, stop=True)
                        nc.vector.tensor_copy(out=denT[:, lt:lt + 1], in_=pdt)
                    nc.vector.reciprocal(out=recipT[h], in_=denT)
                    for lt in range(HMT):
                        nsl = slice(lt * 128, (lt + 1) * 128)
                        for co in range(2):
                            dsb = dsbp.tile([128, 512], F32, tag="dsb")
                            pd = scoreps.tile([128, 512], F32, tag="pscore")
                            nc.tensor.matmul(pd, avT[h][:, nsl],
                                             wo_t[:, co * 512:(co + 1) * 512],
                                             start=True, stop=True)
                            nc.vector.tensor_scalar(
                                out=dsb, in0=pd,
                                scalar1=recipT[h][:, lt:lt + 1], scalar2=None,
                                op0=mybir.AluOpType.mult)
                            nc.gpsimd.dma_start(
                                out=delta_part[h][nsl, co * 512:(co + 1) * 512],
                                in_=dsb)
                    nc.gpsimd.collective_compute(
                        "ReduceScatter", mybir.AluOpType.add, replica_groups=groups,
                        ins=[delta_part[h][:]], outs=[delta_rs[h][:]])

                def phase3_half(h):
                    """y = x + g*(delta+bo) for this half's 128 shard rows."""
                    dr = p3.tile([128, C], F32, tag="dr")
                    nc.gpsimd.dma_start(out=dr, in_=delta_rs[h][:])
                    nc.vector.tensor_add(out=dr, in0=dr, in1=bo_t)
                    gd = p3.tile([128, C], F32, tag=f"gd{h}")
                    nc.vector.tensor_scalar(
                        out=gd, in0=dr, scalar1=g_t[:, h:h + 1],
                        scalar2=None, op0=mybir.AluOpType.mult)
                    for cc in range(2 * CB):
                        xt = xin3.tile([128, 64, HW], F32, tag="xt3")
                        nc.sync.dma_start(
                            out=xt,
                            in_=xs[h * 128:(h + 1) * 128,
                                   cc * 64:(cc + 1) * 64, :])
                        gslice = gd[:, cc * 64:(cc + 1) * 64]
                        nc.vector.tensor_tensor(
                            out=xt, in0=xt,
                            in1=gslice[:, :, None].to_broadcast((128, 64, HW)),
                            op=mybir.AluOpType.add)
                        nc.scalar.dma_start(
                            out=out[h * 128:(h + 1) * 128,
                                    cc * 64:(cc + 1) * 64, :],
                            in_=xt)

                # pipeline order (Tile overlaps across these by dependency)
                load_ft(0)
                qkv_half(0)
                attn_chunk(0)
                load_ft(1)
                qkv_half(1)
                attn_chunk(1)
                finish_half(0)      # RS(A) fires here
                attn_chunk(2)
                attn_chunk(3)
                phase3_half(0)      # streams under attention B + RS(B)
                finish_half(1)
                phase3_half(1)

    _install_wait_split(nc)
    return nc


_NC_CACHE = {}


def get_program(dense=False):
    if dense not in _NC_CACHE:
        _NC_CACHE[dense] = build_program(dense)
    return _NC_CACHE[dense]


def _band_ok(bi_sorted):
    """Check the static band [4j-1, 4j+4] covers every group of each chunk."""
    counts = np.bincount(bi_sorted, minlength=NG)
    if counts.max() > GCAP:
        return False
    s = 0
    for g in range(NG):
        e = s + counts[g]
        if counts[g]:
            for j in range(NQC):
                if s < (j + 1) * 512 and e > j * 512:  # intersects chunk j
                    lo, hi = max(0, 4 * j - 1) * 128, min(MT, 4 * j + 5) * 128
                    if s < lo or e > hi:
                        return False
        s = e
    return True


def _shard_rows(c):
    """Sorted-row indices owned by core c: global tiles c and c+8."""
    return np.r_[128 * c:128 * (c + 1), 1024 + 128 * c:1024 + 128 * (c + 1)]


def prepare_in_maps(x, batch_indices, ln_w, ln_b, in_proj_w, in_proj_b,
                    out_proj_w, out_proj_b, gamma):
    x = np.asarray(x, dtype=np.float32)
    bi_orig = np.asarray(batch_indices).astype(np.int64)
    perm = np.argsort(bi_orig, kind="stable")
    bi = bi_orig[perm]
    dense = not _band_ok(bi)
    ln_w = np.ascontiguousarray(np.asarray(ln_w, np.float32))
    ln_b = np.ascontiguousarray(np.asarray(ln_b, np.float32))
    ipw = np.asarray(in_proj_w, np.float32)
    ipb = np.asarray(in_proj_b, np.float32)
    opw = np.asarray(out_proj_w, np.float32)
    opb = np.ascontiguousarray(np.asarray(out_proj_b, np.float32))
    gamma = np.asarray(gamma, np.float32)

    oh = (bi[None, :] == np.arange(NG, dtype=np.int64)[:, None]).astype(np.float32)
    oha = np.ascontiguousarray((MASK_NEG * oh).astype(ml_dtypes.bfloat16))
    ohb = np.ascontiguousarray((1.0 - oh).astype(ml_dtypes.bfloat16))
    counts = np.bincount(bi, minlength=NG)
    g = np.where(counts[bi] > 1, gamma[0], np.float32(0.0)).astype(np.float32)

    xr = x.reshape(N, C, HW)[perm]
    in_maps = []
    for c in range(NCORES):
        h0 = c * HD
        rows = _shard_rows(c)
        in_maps.append({
            "xs": np.ascontiguousarray(xr[rows]),
            "wq": np.ascontiguousarray(ipw[h0:h0 + HD, :].T.astype(ml_dtypes.bfloat16)),
            "wk": np.ascontiguousarray(ipw[C + h0:C + h0 + HD, :].T.astype(ml_dtypes.bfloat16)),
            "wv": np.ascontiguousarray(ipw[2 * C + h0:2 * C + h0 + HD, :].T.astype(ml_dtypes.bfloat16)),
            "qb": np.ascontiguousarray(ipb[h0:h0 + HD].reshape(HD, 1)),
            "kb": np.ascontiguousarray(ipb[C + h0:C + h0 + HD].reshape(HD, 1)),
            "vb": np.ascontiguousarray(ipb[2 * C + h0:2 * C + h0 + HD].reshape(1, HD).astype(ml_dtypes.bfloat16)),
            "wo": np.ascontiguousarray(opw[:, h0:h0 + HD].T.astype(ml_dtypes.bfloat16)),
            "oha": oha,
            "ohb": ohb,
            "lnw": ln_w,
            "lnb": ln_b,
            "bo": opb,
            "gsh": np.ascontiguousarray(g[rows]),
        })
    return in_maps, perm, dense


def assemble(results, perm):
    y_sorted = np.empty((N, C, 8, 8), np.float32)
    for c in range(NCORES):
        y_sorted[_shard_rows(c)] = results[c]["out"].reshape(NS, C, 8, 8)
    y = np.empty_like(y_sorted)
    y[perm] = y_sorted
    return y


def kernel(**inputs) -> np.ndarray:
    in_maps, perm, dense = prepare_in_maps(**inputs)
    nc = get_program(dense)
    res = run_bass_kernel_spmd(nc, in_maps, list(range(NCORES)), trace=False)
    return assemble(res.results, perm)


# revision 24
# speedup vs baseline: 1.0411x; 1.0411x over previous
"""Trainium2 Bass kernel for nn_BlockCorrelation (sparse/block attention).

Self-contained: accepts FULL inputs, shards across 8 NeuronCores internally,
returns the FULL output.

Math (see reference):
    feat = x.mean((2,3)); feat_n = LN(feat)*ln_w + ln_b
    qkv  = feat_n @ in_proj_w.T + in_proj_b  -> 8-head attention with a
    block-diagonal mask (attend only within equal batch_indices groups)
    out  = attn_out @ out_proj_w.T + out_proj_b
    y    = x + where(group_count>1, gamma*out, 0)[..., None, None]

Distribution / pipelining:
  - rows are sorted by group on the host; core k owns sorted row tiles k and
    k+8 (interleaved). Pooling/LN and the elementwise passes are data-parallel
    over rows; attention is tensor-parallel over heads (core k = head k).
  - per-half pipeline: pool half A -> AllGather featT(A) -> QKV(A) ->
    banded scores/av for query chunks 0-1 -> out_proj partial(A) ->
    ReduceScatter(A) -> the y = x + g*delta pass for half-A rows STREAMS
    while half B's attention and ReduceScatter are still in flight.
  - the sorted band limits each 512-query chunk to key tiles [4j-1, 4j+4]
    (valid while every group fits in one 128 tile; dense fallback otherwise).

Layout: attention tensors are transposed (contraction on partitions); the
softmax denominator is a ones-matmul (no partition reduction); the group mask
is added inside the scores PSUM via a rank-32 one-hot matmul
(-50*(1-same)), so exp() zeroes out-of-group entries.  Scores are O(1)
(LN'd features, 0.02-scale weights), so no max-subtraction is needed.
"""

import json
import sys

if "/opt/trn_rl_repo" not in sys.path:
    sys.path.insert(0, "/opt/trn_rl_repo")

import ml_dtypes
import numpy as np

import concourse.bass as bass
import concourse.mybir as mybir
import concourse.tile as tile
from concourse.bass_utils import run_bass_kernel_spmd
from concourse.masks import make_identity

F32 = mybir.dt.float32
BF16 = mybir.dt.bfloat16

# Problem shape (hardcoded per contract)
N, C, HW = 2048, 1024, 64
NH, HD = 8, 128
NG = 32
EPS = 1e-5
NCORES = 8
NS = N // NCORES          # 256 rows per core
NT = 2                    # halves (core row-tiles per phase)
CB = C // 128             # 8 channel blocks
MT = N // 128             # 16 global key tiles
NQC = N // 512            # 4 query chunks of 512
HMT = MT // 2             # 8 key tiles per half
MASK_NEG = -50.0          # additive mask magnitude (e^-50 ~ 2e-22)
GCAP = 128                # band attention assumes group fits in one m-tile


def _band(j, dense):
    if dense:
        return range(MT)
    return range(max(0, 4 * j - 1), min(MT, 4 * j + 5))


# ---------------------------------------------------------------------------
# walrus workaround: this build rejects >1 sem wait per instruction in some
# CTRL lowerings; split excess on_wait entries onto preceding same-engine
# EventSemaphore instructions (the exact shape wait_ge() lowers to).
def _split_waits_json(j, max_waits=1):
    for f in j.get("functions", []):
        for bb in f.get("blocks", []):
            out = []
            for ins in bb.get("instructions", []):
                si = ins.get("sync_info")
                waits = (si or {}).get("on_wait") or []
                if len(waits) > max_waits:
                    head, tail = waits[:-max_waits], waits[-max_waits:]
                    for k, w in enumerate(head):
                        out.append({
                            "name": f"{ins['name']}-wsplit{k}",
                            "opcode": "EventSemaphore",
                            "engine": ins["engine"],
                            "ins": [],
                            "outs": [],
                            "debug": ins.get("debug", 0),
                            "sync_info": {"on_update": [], "on_wait": [w]},
                        })
                    si["on_wait"] = tail
                out.append(ins)
            bb["instructions"] = out
    return j


def _install_wait_split(nc, max_waits=1):
    def to_json_bytes_fixed():
        j = json.loads(mybir.module_to_json_bytes(nc.m))
        return json.dumps(_split_waits_json(j, max_waits)).encode()

    nc.to_json_bytes = to_json_bytes_fixed


def _bcast_ap(ap, parts=128):
    """DRAM AP broadcast across partitions (stride-0 partition dim)."""
    return bass.AP(tensor=ap.tensor, offset=ap.offset, ap=[[0, parts]] + ap.ap)


# ---------------------------------------------------------------------------
def build_program(dense=False):
    nc = bass.Bass(num_devices=NCORES)

    # --- per-core parameters (SPMD: same program, different data) ---
    # xs rows: [sorted tile k (128 rows), sorted tile k+8 (128 rows)]
    xs = nc.declare_dram_parameter("xs", [NS, C, HW], F32, isOutput=False)
    wq = nc.declare_dram_parameter("wq", [C, HD], BF16, isOutput=False)
    wk = nc.declare_dram_parameter("wk", [C, HD], BF16, isOutput=False)
    wv = nc.declare_dram_parameter("wv", [C, HD], BF16, isOutput=False)
    qb = nc.declare_dram_parameter("qb", [HD, 1], F32, isOutput=False)
    kb = nc.declare_dram_parameter("kb", [HD, 1], F32, isOutput=False)
    vb = nc.declare_dram_parameter("vb", [1, HD], BF16, isOutput=False)
    wo = nc.declare_dram_parameter("wo", [HD, C], BF16, isOutput=False)
    oha = nc.declare_dram_parameter("oha", [NG, N], BF16, isOutput=False)
    ohb = nc.declare_dram_parameter("ohb", [NG, N], BF16, isOutput=False)
    lnw = nc.declare_dram_parameter("lnw", [C], F32, isOutput=False)
    lnb = nc.declare_dram_parameter("lnb", [C], F32, isOutput=False)
    bo = nc.declare_dram_parameter("bo", [C], F32, isOutput=False)
    gsh = nc.declare_dram_parameter("gsh", [NS], F32, isOutput=False)
    out = nc.declare_dram_parameter("out", [NS, C, HW], F32, isOutput=True)

    # --- internal DRAM for collectives (per half) ---
    featT_sh = [nc.dram_tensor(f"featT_sh{h}", [C, 128], BF16) for h in range(NT)]
    featT_full = [
        nc.dram_tensor(f"featT_full{h}", [NCORES * C, 128], BF16,
                       addr_space="Shared")
        for h in range(NT)
    ]
    delta_part = [nc.dram_tensor(f"delta_part{h}", [N // 2, C], F32)
                  for h in range(NT)]
    delta_rs = [nc.dram_tensor(f"delta_rs{h}", [128, C], F32) for h in range(NT)]

    groups = [list(range(NCORES))]
    inv_sqrt_hd = 1.0 / float(np.sqrt(np.float32(HD)))

    with tile.TileContext(nc, num_cores=NCORES) as tc:
        with tc.tile_pool(name="singles", bufs=1) as singles:
            ident = singles.tile([128, 128], F32)
            make_identity(nc, ident)
            ones_col = singles.tile([128, 1], BF16)
            nc.vector.memset(ones_col, 1.0)
            one_1x1 = singles.tile([1, 1], F32)
            nc.vector.memset(one_1x1, 1.0)
            ones_row = singles.tile([1, 128], BF16)
            nc.vector.memset(ones_row, 1.0)

            # ------------- phase 1: pool + LN + transpose + AG per half -------------
            with (
                tc.tile_pool(name="xin", bufs=3) as xin,
                tc.tile_pool(name="p1", bufs=2) as p1,
                tc.tile_pool(name="p1one", bufs=1) as p1one,
                tc.tile_pool(name="p1ps", bufs=2, space="PSUM") as p1ps,
            ):
                eps_t = p1one.tile([128, 1], F32)
                nc.vector.memset(eps_t, EPS * HW * HW)  # LN on sums: eps * 64^2
                lnw_t = p1one.tile([128, C], F32)
                nc.gpsimd.dma_start(out=lnw_t, in_=_bcast_ap(lnw[:]))
                lnb_t = p1one.tile([128, C], F32)
                nc.gpsimd.dma_start(out=lnb_t, in_=_bcast_ap(lnb[:]))

                for h in range(NT):
                    fsum = p1.tile([128, C], F32, tag="fsum")
                    for cc in range(CB):
                        xt = xin.tile([128, 128, HW], F32, tag="xt")
                        nc.scalar.dma_start(
                            out=xt,
                            in_=xs[h * 128:(h + 1) * 128,
                                   cc * 128:(cc + 1) * 128, :])
                        nc.vector.reduce_sum(
                            out=fsum[:, cc * 128:(cc + 1) * 128],
                            in_=xt, axis=mybir.AxisListType.X)
                    stats = p1.tile([128, 2, 6], F32, tag="stats")
                    for sg in range(2):
                        nc.vector.bn_stats(out=stats[:, sg, :],
                                           in_=fsum[:, sg * 512:(sg + 1) * 512])
                    mv = p1.tile([128, 2], F32, tag="mv")
                    nc.vector.bn_aggr(out=mv, in_=stats)
                    std = p1.tile([128, 1], F32, tag="std")
                    nc.scalar.activation(
                        out=std, in_=mv[:, 1:2],
                        func=mybir.ActivationFunctionType.Sqrt, bias=eps_t, scale=1.0)
                    rstd = p1.tile([128, 1], F32, tag="rstd")
                    nc.vector.reciprocal(out=rstd, in_=std)
                    featn = p1.tile([128, C], F32, tag="featn")
                    nc.vector.tensor_scalar(
                        out=featn, in0=fsum, scalar1=mv[:, 0:1], scalar2=rstd,
                        op0=mybir.AluOpType.subtract, op1=mybir.AluOpType.mult)
                    nc.vector.tensor_mul(out=featn, in0=featn, in1=lnw_t)
                    nc.vector.tensor_add(out=featn, in0=featn, in1=lnb_t)
                    fTb = p1.tile([128, CB, 128], BF16, tag="fTb")
                    for cbi in range(CB):
                        pt = p1ps.tile([128, 128], F32, tag="trps")
                        nc.tensor.transpose(pt, featn[:, cbi * 128:(cbi + 1) * 128],
                                            ident)
                        nc.vector.tensor_copy(out=fTb[:, cbi, :], in_=pt)
                    nc.sync.dma_start(
                        out=featT_sh[h].rearrange("(cb p) n -> p cb n", p=128),
                        in_=fTb)
                    nc.gpsimd.collective_compute(
                        "AllGather", mybir.AluOpType.bypass, replica_groups=groups,
                        ins=[featT_sh[h][:]], outs=[featT_full[h][:]])

            # ---------------- phase 2+3: per-half pipeline ----------------
            with (
                tc.tile_pool(name="attper", bufs=1) as attper,
                tc.tile_pool(name="qkvw", bufs=1) as qkvw,
                tc.tile_pool(name="maskp", bufs=1) as maskp,
                tc.tile_pool(name="expbuf", bufs=2) as expbuf,
                tc.tile_pool(name="dsbp", bufs=2) as dsbp,
                tc.tile_pool(name="p3", bufs=1) as p3,
                tc.tile_pool(name="ftp", bufs=1) as ftp,
                tc.tile_pool(name="xin3", bufs=6) as xin3,
                tc.tile_pool(name="qkvps", bufs=2, space="PSUM") as qkvps,
                tc.tile_pool(name="scoreps", bufs=3, space="PSUM") as scoreps,
                tc.tile_pool(name="avdps", bufs=2, space="PSUM") as avdps,
                tc.tile_pool(name="denps", bufs=1, space="PSUM") as denps,
            ):
                # head weights + onehots (tiny; load once)
                wq_t = qkvw.tile([128, CB, HD], BF16)
                nc.sync.dma_start(out=wq_t,
                                  in_=wq.rearrange("(cb p) d -> p cb d", p=128))
                wk_t = qkvw.tile([128, CB, HD], BF16)
                nc.sync.dma_start(out=wk_t,
                                  in_=wk.rearrange("(cb p) d -> p cb d", p=128))
                wv_t = qkvw.tile([128, CB, HD], BF16)
                nc.sync.dma_start(out=wv_t,
                                  in_=wv.rearrange("(cb p) d -> p cb d", p=128))
                qb_t = qkvw.tile([128, 1], F32)
                nc.sync.dma_start(out=qb_t, in_=qb[:])
                kb_t = qkvw.tile([128, 1], F32)
                nc.sync.dma_start(out=kb_t, in_=kb[:])
                vb_t = qkvw.tile([1, 128], BF16)
                nc.sync.dma_start(out=vb_t, in_=vb[:])
                wo_t = qkvw.tile([128, C], BF16)
                nc.sync.dma_start(out=wo_t, in_=wo[:])
                oha_t = maskp.tile([128, N], BF16)
                nc.vector.memset(oha_t, 0.0)
                nc.sync.dma_start(out=oha_t[:NG, :], in_=oha[:])
                ohb_t = maskp.tile([128, N], BF16)
                nc.vector.memset(ohb_t, 0.0)
                nc.sync.dma_start(out=ohb_t[:NG, :], in_=ohb[:])
                bo_t = p3.tile([128, C], F32)
                nc.gpsimd.dma_start(out=bo_t, in_=_bcast_ap(bo[:]))
                g_t = p3.tile([128, NT], F32)
                nc.gpsimd.dma_start(out=g_t, in_=gsh.rearrange("(t p) -> p t", p=128))

                # per-half feature/QKV tensors (global n = 1024*h + 128*co + nl)
                ft = {}
                qT = [attper.tile([128, N // 2], BF16, tag=f"qT{h}", name=f"qT{h}")
                      for h in range(NT)]
                kT = [attper.tile([128, N // 2], BF16, tag=f"kT{h}", name=f"kT{h}")
                      for h in range(NT)]
                v_t = [attper.tile([128, HMT, HD], BF16, tag=f"v{h}", name=f"v{h}")
                       for h in range(NT)]
                avT = [attper.tile([128, N // 2], BF16, tag=f"avT{h}", name=f"avT{h}")
                       for h in range(NT)]
                den_row = [attper.tile([1, N // 2], F32, tag=f"den{h}", name=f"den{h}")
                           for h in range(NT)]
                recipT = [attper.tile([128, HMT], F32, tag=f"recipT{h}", name=f"recipT{h}")
                          for h in range(NT)]

                def load_ft(h):
                    ft[h] = ftp.tile([128, CB, N // 2], BF16, tag="ft", name="ft")
                    ftv = featT_full[h].rearrange(
                        "(co cb p) n -> cb p co n", co=NCORES, p=128)
                    for cbi in range(CB):
                        nc.sync.dma_start(
                            out=ft[h][:, cbi, :].rearrange(
                                "p (co nl) -> p co nl", co=NCORES),
                            in_=ftv[cbi])

                def qkv_half(h):
                    for jj in range(2):
                        sl = slice(jj * 512, (jj + 1) * 512)
                        pq = qkvps.tile([128, 512], F32, tag="pqkv")
                        for cbi in range(CB):
                            nc.tensor.matmul(pq, wq_t[:, cbi, :], ft[h][:, cbi, sl],
                                             start=(cbi == 0), stop=(cbi == CB - 1))
                        nc.vector.tensor_scalar(
                            out=qT[h][:, sl], in0=pq, scalar1=qb_t,
                            scalar2=inv_sqrt_hd,
                            op0=mybir.AluOpType.add, op1=mybir.AluOpType.mult)
                        pk = qkvps.tile([128, 512], F32, tag="pqkv")
                        for cbi in range(CB):
                            nc.tensor.matmul(pk, wk_t[:, cbi, :], ft[h][:, cbi, sl],
                                             start=(cbi == 0), stop=(cbi == CB - 1))
                        nc.vector.tensor_scalar(
                            out=kT[h][:, sl], in0=pk, scalar1=kb_t, scalar2=None,
                            op0=mybir.AluOpType.add)
                    for lt in range(HMT):
                        msl = slice(lt * 128, (lt + 1) * 128)
                        pv = qkvps.tile([128, HD], F32, tag="pqkv")
                        for cbi in range(CB):
                            nc.tensor.matmul(pv, ft[h][:, cbi, msl], wv_t[:, cbi, :],
                                             start=(cbi == 0), stop=False)
                        nc.tensor.matmul(pv, ones_row, vb_t, start=False, stop=True)
                        nc.scalar.activation(out=v_t[h][:, lt, :], in_=pv,
                                             func=mybir.ActivationFunctionType.Copy)

                BW = MT if dense else 6

                def attn_chunk(j):
                    """scoresT band -> exp -> av + denominator for query chunk j."""
                    h = j // 2
                    sl = slice((j % 2) * 512, (j % 2 + 1) * 512)
                    gsl = slice(j * 512, (j + 1) * 512)  # global (onehot) cols
                    band = list(_band(j, dense))
                    expj = expbuf.tile([128, BW, 512], BF16, tag="expj")
                    pav = avdps.tile([128, 512], F32, tag="pav")
                    pden = denps.tile([1, 512], F32, tag="pden")
                    for bi_, mt in enumerate(band):
                        mh, lt = mt // HMT, mt % HMT
                        msl = slice(lt * 128, (lt + 1) * 128)
                        gmsl = slice(mt * 128, (mt + 1) * 128)
                        ps = scoreps.tile([128, 512], F32, tag="pscore")
                        nc.tensor.matmul(ps, kT[mh][:, msl], qT[h][:, sl],
                                         start=True, stop=False)
                        nc.tensor.matmul(ps, oha_t[:, gmsl], ohb_t[:, gsl],
                                         start=False, stop=True)
                        nc.scalar.activation(
                            out=expj[:, bi_, :], in_=ps,
                            func=mybir.ActivationFunctionType.Exp)
                    nb = len(band)
                    for bi_, mt in enumerate(band):
                        mh, lt = mt // HMT, mt % HMT
                        nc.tensor.matmul(pav, v_t[mh][:, lt, :], expj[:, bi_, :],
                                         start=(bi_ == 0), stop=(bi_ == nb - 1))
                    for bi_, mt in enumerate(band):
                        nc.tensor.matmul(pden, ones_col, expj[:, bi_, :],
                                         start=(bi_ == 0), stop=(bi_ == nb - 1))
                    nc.vector.tensor_copy(out=avT[h][:, sl], in_=pav)
                    nc.vector.tensor_copy(out=den_row[h][:, sl], in_=pden)

                def finish_half(h):
                    """denominators -> out_proj partial -> ReduceScatter."""
                    denT = attper.tile([128, HMT], F32, tag=f"denT{h}", name=f"denT{h}")
                    for lt in range(HMT):
                        pdt = qkvps.tile([128, 1], F32, tag="pqkv")
                        nc.tensor.matmul(pdt,
                                         den_row[h][0:1, lt * 128:(lt + 1) * 128],
                                         one_1x1, start=True, stop=True)
                        nc.vector.tensor_copy(out=denT[:, lt:lt + 1], in_=pdt)
                    nc.vector.reciprocal(out=recipT[h], in_=denT)
                    for lt in range(HMT):
                        nsl = slice(lt * 128, (lt + 1) * 128)
                        for co in range(2):
                            dsb = dsbp.tile([128, 512], F32, tag="dsb")
                            pd = scoreps.tile([128, 512], F32, tag="pscore")
                            nc.tensor.matmul(pd, avT[h][:, nsl],
                                             wo_t[:, co * 512:(co + 1) * 512],
                                             start=True, stop=True)
                            nc.vector.tensor_scalar(
                                out=dsb, in0=pd,
                                scalar1=recipT[h][:, lt:lt + 1], scalar2=None,
                                op0=mybir.AluOpType.mult)
                            nc.scalar.dma_start(
                                out=delta_part[h][nsl, co * 512:(co + 1) * 512],
                                in_=dsb)
                    nc.gpsimd.collective_compute(
                        "ReduceScatter", mybir.AluOpType.add, replica_groups=groups,
                        ins=[delta_part[h][:]], outs=[delta_rs[h][:]])

                def phase3_half(h):
                    """y = x + g*(delta+bo) for this half's 128 shard rows."""
                    dr = p3.tile([128, C], F32, tag="dr")
                    nc.gpsimd.dma_start(out=dr, in_=delta_rs[h][:])
                    nc.vector.tensor_add(out=dr, in0=dr, in1=bo_t)
                    gd = p3.tile([128, C], F32, tag=f"gd{h}")
                    nc.vector.tensor_scalar(
                        out=gd, in0=dr, scalar1=g_t[:, h:h + 1],
                        scalar2=None, op0=mybir.AluOpType.mult)
                    for cc in range(2 * CB):
                        xt = xin3.tile([128, 64, HW], F32, tag="xt3")
                        nc.sync.dma_start(
                            out=xt,
                            in_=xs[h * 128:(h + 1) * 128,
                                   cc * 64:(cc + 1) * 64, :])
                        gslice = gd[:, cc * 64:(cc + 1) * 64]
                        nc.vector.tensor_tensor(
                            out=xt, in0=xt,
                            in1=gslice[:, :, None].to_broadcast((128, 64, HW)),
                            op=mybir.AluOpType.add)
                        nc.scalar.dma_start(
                            out=out[h * 128:(h + 1) * 128,
                                    cc * 64:(cc + 1) * 64, :],
                            in_=xt)

                # pipeline order (Tile overlaps across these by dependency)
                load_ft(0)
                qkv_half(0)
                attn_chunk(0)
                load_ft(1)
                qkv_half(1)
                attn_chunk(1)
                finish_half(0)      # RS(A) fires here
                attn_chunk(2)
                attn_chunk(3)
                phase3_half(0)      # streams under attention B + RS(B)
                finish_half(1)
                phase3_half(1)

    _install_wait_split(nc)
    return nc


_NC_CACHE = {}


def get_program(dense=False):
    if dense not in _NC_CACHE:
        _NC_CACHE[dense] = build_program(dense)
    return _NC_CACHE[dense]


def _band_ok(bi_sorted):
    """Check the static band [4j-1, 4j+4] covers every group of each chunk."""
    counts = np.bincount(bi_sorted, minlength=NG)
    if counts.max() > GCAP:
        return False
    s = 0
    for g in range(NG):
        e = s + counts[g]
        if counts[g]:
            for j in range(NQC):
                if s < (j + 1) * 512 and e > j * 512:  # intersects chunk j
                    lo, hi = max(0, 4 * j - 1) * 128, min(MT, 4 * j + 5) * 128
                    if s < lo or e > hi:
                        return False
        s = e
    return True


def _shard_rows(c):
    """Sorted-row indices owned by core c: global tiles c and c+8."""
    return np.r_[128 * c:128 * (c + 1), 1024 + 128 * c:1024 + 128 * (c + 1)]


def prepare_in_maps(x, batch_indices, ln_w, ln_b, in_proj_w, in_proj_b,
                    out_proj_w, out_proj_b, gamma):
    x = np.asarray(x, dtype=np.float32)
    bi_orig = np.asarray(batch_indices).astype(np.int64)
    perm = np.argsort(bi_orig, kind="stable")
    bi = bi_orig[perm]
    dense = not _band_ok(bi)
    ln_w = np.ascontiguousarray(np.asarray(ln_w, np.float32))
    ln_b = np.ascontiguousarray(np.asarray(ln_b, np.float32))
    ipw = np.asarray(in_proj_w, np.float32)
    ipb = np.asarray(in_proj_b, np.float32)
    opw = np.asarray(out_proj_w, np.float32)
    opb = np.ascontiguousarray(np.asarray(out_proj_b, np.float32))
    gamma = np.asarray(gamma, np.float32)

    oh = (bi[None, :] == np.arange(NG, dtype=np.int64)[:, None]).astype(np.float32)
    oha = np.ascontiguousarray((MASK_NEG * oh).astype(ml_dtypes.bfloat16))
    ohb = np.ascontiguousarray((1.0 - oh).astype(ml_dtypes.bfloat16))
    counts = np.bincount(bi, minlength=NG)
    g = np.where(counts[bi] > 1, gamma[0], np.float32(0.0)).astype(np.float32)

    xr = x.reshape(N, C, HW)[perm]
    in_maps = []
    for c in range(NCORES):
        h0 = c * HD
        rows = _shard_rows(c)
        in_maps.append({
            "xs": np.ascontiguousarray(xr[rows]),
            "wq": np.ascontiguousarray(ipw[h0:h0 + HD, :].T.astype(ml_dtypes.bfloat16)),
            "wk": np.ascontiguousarray(ipw[C + h0:C + h0 + HD, :].T.astype(ml_dtypes.bfloat16)),
            "wv": np.ascontiguousarray(ipw[2 * C + h0:2 * C + h0 + HD, :].T.astype(ml_dtypes.bfloat16)),
            "qb": np.ascontiguousarray(ipb[h0:h0 + HD].reshape(HD, 1)),
            "kb": np.ascontiguousarray(ipb[C + h0:C + h0 + HD].reshape(HD, 1)),
            "vb": np.ascontiguousarray(ipb[2 * C + h0:2 * C + h0 + HD].reshape(1, HD).astype(ml_dtypes.bfloat16)),
            "wo": np.ascontiguousarray(opw[:, h0:h0 + HD].T.astype(ml_dtypes.bfloat16)),
            "oha": oha,
            "ohb": ohb,
            "lnw": ln_w,
            "lnb": ln_b,
            "bo": opb,
            "gsh": np.ascontiguousarray(g[rows]),
        })
    return in_maps, perm, dense


def assemble(results, perm):
    y_sorted = np.empty((N, C, 8, 8), np.float32)
    for c in range(NCORES):
        y_sorted[_shard_rows(c)] = results[c]["out"].reshape(NS, C, 8, 8)
    y = np.empty_like(y_sorted)
    y[perm] = y_sorted
    return y


def kernel(**inputs) -> np.ndarray:
    in_maps, perm, dense = prepare_in_maps(**inputs)
    nc = get_program(dense)
    res = run_bass_kernel_spmd(nc, in_maps, list(range(NCORES)), trace=False)
    return assemble(res.results, perm)
